# revision 7
# baseline (speedup 1.0000x reference)
"""FLGC (fused learned group conv) forward for Trainium2, 8-core data parallel.

The reference collapses to:  out[b, j, hw] = sum_c W[j, c] * x[b, c, hw]
where W folds the softmax gates, group mask, s/t gains, and the double
output permutation:
    W = (conv[:,:,0,0] * t_gain[:,None] * mask * s_gain[None,:])[p[p], :]

W is group-block-sparse: row j only reads channels c with s[c] == t[p[p][j]].
On the host we pick ONE ordering of the 16 groups (searched to minimize
work), sort input channels and output channels by it, and cut both sides
into four dense 128-channel blocks. In that ordering the sorted W is banded:
only ~6 of the 16 (in-block, out-block) weight blocks are nonzero.

The device kernel is a dense blocked matmul over the nonzero blocks, and the
whole datapath is bf16: x is downcast on the host, streamed in bf16, matmuls
accumulate in fp32 PSUM, and the output is written back as bf16 then upcast
on the host (max rel err ~1e-3, well inside the 2e-2 gate). That halves HBM
traffic — this kernel is memory-bound, so bf16 is worth ~2x by itself — and
runs the PE at full rate (fp32 matmul is 1/4 rate).

DMA plan: channels are packed host-side as (128, KB) so one dma_start moves
a full 128-partition x tile (128 x 4 x 3072 bf16 = 3.1 MB, line-rate sized);
same for the output. Loads issue on SP and stores on ACT — the two HWDGE
rings — so a store's semaphore wait never head-of-line blocks the next
tile's load. Batch is sharded 2 images per core across 8 cores.

Fallback (pathological gate distributions where the band search stays wide):
dense 512x512 fp32r (TF32) matmul — also memory-bound, ~2e-4 max rel err.
"""

import os

import ml_dtypes
import numpy as np

import concourse.bacc as bacc
import concourse.bass as bass
import concourse.mybir as mybir
import concourse.tile as tile
from concourse.bass import ds
from concourse.bass_utils import run_bass_kernel_spmd

# NTFF tracing is not reachable through the axon tunnel in this container
# (antenv.axon_hooks absent); a stray BASS_TRACE=1 would crash the run.
os.environ["BASS_NEVER_TRACE"] = "1"

# Problem shapes (hardcoded per harness contract)
B, C, H, W_SP = 16, 512, 96, 96
G = 16
HW = H * W_SP            # 9216
N_CORES = 8
B_LOC = B // N_CORES     # 2
KB = C // 128            # 4 input-channel blocks
MB = C // 128            # 4 output-channel blocks
N_DMA = 3072             # spatial columns per DMA tile (3.1 MB per transfer)
N_MM = 512               # spatial columns per matmul (one fp32 PSUM bank)
SUB = N_DMA // N_MM      # 6
NT = HW // N_DMA         # 3 DMA tiles per image

BAND_THRESH = 9          # use banded kernel if nonzero block pairs <= this

F32 = mybir.dt.float32
F32R = mybir.dt.float32r
BF16 = mybir.dt.bfloat16
NP_BF16 = ml_dtypes.bfloat16

LAST_RESULT = None       # BassKernelResults of the most recent run (for test.py)
_NC_CACHE = {}


def _build_nc_banded(pairs, loop=None, unroll=8):
    """pairs: ordered tuple of (in_block, out_block) nonzero weight blocks.

    x/y are in band-sorted channel order, packed host-side per DMA tile:
    x[b, p, t, i*N_DMA + n] = sorted-channel (i*128+p) at column t*N_DMA+n,
    so one dma_start moves a full tile as one contiguous 24 KB run per
    partition (128 max-size descriptors). wt[p] is the lhsT (k, m) 128x128
    bf16 block for pairs[p]. loop=R wraps R repeats of the body in a For_i
    hardware loop (used by test.py's repeat-scaling timer), unrolled
    `unroll` bodies per iteration to amortize For_i's all-engine barrier.
    """
    np_ = len(pairs)
    nc = bacc.Bacc("TRN2", target_bir_lowering=False, debug=False)
    x_d = nc.dram_tensor(
        "x", (B_LOC, 128, NT, KB * N_DMA), BF16, kind="ExternalInput"
    )
    wt_d = nc.dram_tensor("wt", (np_, 128, 128), BF16, kind="ExternalInput")
    y_d = nc.dram_tensor(
        "y", (B_LOC, 128, NT, MB * N_DMA), BF16, kind="ExternalOutput"
    )

    by_out = [[] for _ in range(MB)]
    for idx, (i, j) in enumerate(pairs):
        by_out[j].append((idx, i))

    with tile.TileContext(nc) as tc:
        with (
            tc.tile_pool(name="wt", bufs=1) as wt_pool,
            tc.tile_pool(name="xin", bufs=3) as x_pool,
            tc.tile_pool(name="out", bufs=3) as o_pool,
            tc.tile_pool(name="ps", bufs=8, space=bass.MemorySpace.PSUM) as ps_pool,
        ):
            wt_sb = wt_pool.tile([128, np_, 128], BF16)
            for p in range(np_):
                nc.sync.dma_start(wt_sb[:, p, :], wt_d[p])

            def body():
                for b in range(B_LOC):
                    for t in range(NT):
                        x_sb = x_pool.tile([128, KB * N_DMA], BF16, tag="x_sb")
                        nc.sync.dma_start(x_sb[:], x_d[b, :, t])
                        o_sb = o_pool.tile([128, MB * N_DMA], BF16, tag="o_sb")
                        for sub in range(SUB):
                            for m0 in range(MB):
                                ps = ps_pool.tile([128, N_MM], F32, tag="ps")
                                blocks = by_out[m0]
                                for n, (idx, i) in enumerate(blocks):
                                    nc.tensor.matmul(
                                        ps[:],
                                        wt_sb[:, idx, :],
                                        x_sb[:, ds(i * N_DMA + sub * N_MM, N_MM)],
                                        start=(n == 0),
                                        stop=(n == len(blocks) - 1),
                                    )
                                nc.vector.tensor_copy(
                                    o_sb[:, ds(m0 * N_DMA + sub * N_MM, N_MM)],
                                    ps[:],
                                )
                        nc.scalar.dma_start(y_d[b, :, t], o_sb[:])

            if loop is None:
                body()
            else:
                assert loop % unroll == 0
                with tc.For_i(0, loop // unroll, 1):
                    for _ in range(unroll):
                        body()
    nc.compile()
    return nc


def _build_nc_dense(repeat=1):
    """Dense fp32r (TF32) fallback; plain (B_LOC, C, HW) layouts."""
    nc = bacc.Bacc("TRN2", target_bir_lowering=False, debug=False)
    x_d = nc.dram_tensor("x", (B_LOC, C, HW), F32, kind="ExternalInput")
    wt_d = nc.dram_tensor("wt", (128, KB, MB, 128), F32, kind="ExternalInput")
    y_d = nc.dram_tensor("y", (B_LOC, C, HW), F32, kind="ExternalOutput")

    ND, NS = 1536, 1536 // 512

    with tile.TileContext(nc) as tc:
        with (
            tc.tile_pool(name="wt", bufs=1) as wt_pool,
            tc.tile_pool(name="xin", bufs=3) as x_pool,
            tc.tile_pool(name="out", bufs=3) as o_pool,
            tc.tile_pool(name="ps", bufs=8, space=bass.MemorySpace.PSUM) as ps_pool,
        ):
            wt_sb = wt_pool.tile([128, KB, MB, 128], F32R)
            nc.sync.dma_start(wt_sb[:], wt_d[:].bitcast(F32R))

            def body():
                for b in range(B_LOC):
                    for t in range(HW // ND):
                        x_sb = x_pool.tile([128, KB, ND], F32R, tag="x_sb")
                        for k0 in range(KB):
                            nc.sync.dma_start(
                                x_sb[:, k0, :],
                                x_d[
                                    b, k0 * 128 : (k0 + 1) * 128, ds(t * ND, ND)
                                ].bitcast(F32R),
                            )
                        o_sb = o_pool.tile([128, MB, ND], F32, tag="o_sb")
                        for sub in range(NS):
                            for m0 in range(MB):
                                ps = ps_pool.tile([128, 512], F32, tag="ps")
                                for k0 in range(KB):
                                    nc.tensor.matmul(
                                        ps[:],
                                        wt_sb[:, k0, m0, :],
                                        x_sb[:, k0, ds(sub * 512, 512)],
                                        start=(k0 == 0),
                                        stop=(k0 == KB - 1),
                                    )
                                nc.vector.tensor_copy(
                                    o_sb[:, m0, ds(sub * 512, 512)], ps[:]
                                )
                        for m0 in range(MB):
                            nc.scalar.dma_start(
                                y_d[
                                    b, m0 * 128 : (m0 + 1) * 128, ds(t * ND, ND)
                                ],
                                o_sb[:, m0, :],
                            )

            for _ in range(repeat):
                body()
    nc.compile()
    return nc


def _softmax(a):
    a = a - a.max(axis=1, keepdims=True)
    e = np.exp(a)
    return e / e.sum(axis=1, keepdims=True)


def _gates(conv, S, T):
    """Replicate the reference's gate math; return folded W plus group ids."""
    s_hat = _softmax(S.astype(np.float32))
    t_hat = _softmax(T.astype(np.float32))
    s = s_hat.argmax(axis=1)
    t = t_hat.argmax(axis=1)
    c_in, c_out = S.shape[0], T.shape[0]
    s_gain = s_hat[np.arange(c_in), s]
    t_gain = t_hat[np.arange(c_out), t]
    mask = (t[:, None] == s[None, :]).astype(np.float32)
    w_eff = conv[:, :, 0, 0] * t_gain[:, None] * mask
    p = np.argsort(t, kind="stable")
    pp = p[p]
    W = (w_eff * s_gain[None, :])[pp, :].astype(np.float32)
    gfin = t[pp]  # group id of each final output channel
    return W, s, gfin


def _count_pairs(order, ins, outs):
    pairs = set()
    icum = ocum = 0
    for g in order:
        if ins[g] or outs[g]:
            i0 = icum // 128
            i1 = (icum + max(ins[g], 1) - 1) // 128
            o0 = ocum // 128
            o1 = (ocum + max(outs[g], 1) - 1) // 128
            pairs.update(
                (i, o) for i in range(i0, i1 + 1) for o in range(o0, o1 + 1)
            )
        icum += ins[g]
        ocum += outs[g]
    return pairs


def _find_band_order(s, gfin, trials=60000):
    """Search a group ordering minimizing nonzero (in,out) weight blocks."""
    ins = np.bincount(s, minlength=G)
    outs = np.bincount(gfin, minlength=G)
    rng = np.random.default_rng(12345)
    order = np.arange(G)
    best_p, best_o = len(_count_pairs(order, ins, outs)), order.copy()
    for _ in range(trials):
        rng.shuffle(order)
        p = len(_count_pairs(order, ins, outs))
        if p < best_p:
            cur = order.copy()
            improved = True
            while improved:
                improved = False
                for a in range(G):
                    for b_ in range(a + 1, G):
                        cur[a], cur[b_] = cur[b_], cur[a]
                        q = len(_count_pairs(cur, ins, outs))
                        if q < p:
                            p = q
                            improved = True
                        else:
                            cur[a], cur[b_] = cur[b_], cur[a]
            best_p, best_o = p, cur.copy()
        if best_p <= 6:
            break
    return best_o, sorted(_count_pairs(best_o, ins, outs))


def _prep_banded(x, conv, S, T):
    """Host-side prep for the banded path. Returns None if the band search
    stays too wide (caller falls back to the dense kernel)."""
    W, s, gfin = _gates(conv, S, T)
    order, pairs = _find_band_order(s, gfin)
    if len(pairs) > BAND_THRESH:
        return None
    pairs = tuple(pairs)
    in_order = np.concatenate([np.nonzero(s == g)[0] for g in order])
    out_order = np.concatenate([np.nonzero(gfin == g)[0] for g in order])
    W_sorted = W[np.ix_(out_order, in_order)]

    wt = np.empty((len(pairs), 128, 128), dtype=NP_BF16)
    for p, (i, j) in enumerate(pairs):
        wt[p] = W_sorted[j * 128 : (j + 1) * 128, i * 128 : (i + 1) * 128].T

    # pack channels (p, k): partition p of block k = sorted channel k*128+p,
    # then per DMA tile t so each tile is contiguous per partition
    idx2d = in_order.reshape(KB, 128).T
    x_pre = (
        x.reshape(B, C, NT, N_DMA)[:, idx2d]           # (B, 128, KB, NT, N_DMA)
        .transpose(0, 1, 3, 2, 4)
        .astype(NP_BF16, order="C")
        .reshape(B, 128, NT, KB * N_DMA)
    )
    return pairs, wt, x_pre, out_order


def kernel(x, conv, S, T):
    global LAST_RESULT
    x = np.ascontiguousarray(np.asarray(x, dtype=np.float32))
    conv = np.asarray(conv, dtype=np.float32)
    S = np.asarray(S, dtype=np.float32)
    T = np.asarray(T, dtype=np.float32)

    prep = _prep_banded(x, conv, S, T)
    if prep is not None:
        pairs, wt, x_pre, out_order = prep

        key = ("banded16", pairs)
        if key not in _NC_CACHE:
            _NC_CACHE.clear()
            _NC_CACHE[key] = _build_nc_banded(pairs)
        nc = _NC_CACHE[key]

        in_maps = [
            {"x": x_pre[i * B_LOC : (i + 1) * B_LOC], "wt": wt}
            for i in range(N_CORES)
        ]
        res = run_bass_kernel_spmd(nc, in_maps, core_ids=list(range(N_CORES)))
        LAST_RESULT = res

        y_all = np.concatenate(
            [np.asarray(r["y"]) for r in res.results]
        )  # (B, 128, NT, MB*N_DMA) bf16
        y_sorted = (
            y_all.reshape(B, 128, NT, MB, N_DMA)
            .transpose(0, 3, 1, 2, 4)                  # (B, MB, 128, NT, N_DMA)
            .astype(np.float32, order="C")
            .reshape(B, C, HW)
        )
        out = np.empty((B, C, HW), dtype=np.float32)
        out[:, out_order] = y_sorted
        return np.ascontiguousarray(out.reshape(B, C, H, W_SP))

    # Fallback: dense fp32r
    W, _, _ = _gates(conv, S, T)
    key = ("dense",)
    if key not in _NC_CACHE:
        _NC_CACHE.clear()
        _NC_CACHE[key] = _build_nc_dense()
    nc = _NC_CACHE[key]
    wt = np.ascontiguousarray(W.T.reshape(KB, 128, MB, 128).transpose(1, 0, 2, 3))
    in_maps = [
        {"x": x[i * B_LOC : (i + 1) * B_LOC].reshape(B_LOC, C, HW), "wt": wt}
        for i in range(N_CORES)
    ]
    res = run_bass_kernel_spmd(nc, in_maps, core_ids=list(range(N_CORES)))
    LAST_RESULT = res
    out = np.empty((B, C, H, W_SP), dtype=np.float32)
    for i, r in enumerate(res.results):
        out[i * B_LOC : (i + 1) * B_LOC] = r["y"].reshape(B_LOC, C, H, W_SP)
    return out
